# revision 9
# baseline (speedup 1.0000x reference)
"""Trainium2 Bass kernel for the CP-sparse-degree-LU module.

Reference computation (all fp32):
    zf  = z.reshape(-1, 2048)                      # [N=8192, d]
    W   = masks * U                                # [6, k, d]
    out = zf @ W[0].T                              # [N, k]
    for i in 1..5: out = (zf @ W[i].T) * out + out
    x   = out @ C_w.T + C_b                        # [N, o]

Sharding: data-parallel over the token dim N across 8 cores (1024 tokens
each), weights replicated; no collectives. Everything is laid out
transposed on device (acc is [k, tok], output is [o, tok]) so the degree
chain and the final projection both run without on-device transposes:
    acc.T = W_i @ z.T  -> lhsT = W_i.T tiles [d,k], rhs = z.T [d, tok]
    x.T   = C_w @ acc  -> lhsT = C_w.T tiles [k,o], rhs = acc [k, tok]

Sparsity: W = masks*U is block-sparse (tril/triu factors plus a degree
mask that zeroes rank rows < i*K/DEGREE at degree i). The host detects
all-zero 128x128 blocks of the actual W at runtime and builds the device
program skipping them: a skipped (degree, rank-tile) group contributes
mm = 0, so acc = (0+1)*acc is the identity and the whole group (DMA,
matmuls, DVE update) is dropped. This is sound for arbitrary inputs —
only provably-zero blocks are skipped; dense inputs yield the dense
program.

Matmuls run in float32r (fp32 bits read FP22-truncated, single-pass full
rate vs 4-pass true fp32); the chain update acc = (mm + 1) * acc is one
DVE scalar_tensor_tensor op reading the PSUM accumulation directly.
"""

import os
import sys
import types
from contextlib import ExitStack

import numpy as np

DEGREE, D, K, O = 6, 2048, 2048, 2048
N_CORES = 8
N_TOTAL = 8192
TOK = N_TOTAL // N_CORES  # 1024 tokens per core
P = 128
DT = D // P  # 16 contraction tiles (degree matmuls)
KT = K // P  # 16 rank tiles
OT = O // P  # 16 output tiles
NC_CHUNK = 512  # moving free dim per matmul (PSUM bank, fp32 max)
TC = TOK // NC_CHUNK  # 2 token chunks

_CACHE = {}


def _install_ntff_shim():
    """Register antenv.axon_hooks so run_bass_kernel_spmd(trace=True) can
    profile under axon. Safe no-op if anything is unavailable."""
    try:
        if "antenv.axon_hooks" in sys.modules:
            return
        mod = types.ModuleType("antenv.axon_hooks")
        mod._hook = None
        mod.set_axon_ntff_profile_hook = lambda h: setattr(mod, "_hook", h)
        mod.get_axon_ntff_profile_hook = lambda: mod._hook
        sys.modules["antenv.axon_hooks"] = mod
        from trn_agent_boot.trn_boot import _ntff_profile_via_ctypes

        mod._hook = _ntff_profile_via_ctypes("/opt/axon/libaxon_pjrt.so")
    except Exception:
        pass


def _build(ranges):
    """ranges[i][kt] = (dt_lo, dt_hi) inclusive active range, or None if the
    whole (degree, rank-tile) block row is zero."""
    import concourse.tile as tile
    from concourse import bacc, mybir

    f32 = mybir.dt.float32
    f32r = mybir.dt.float32r
    ADD = mybir.AluOpType.add
    MULT = mybir.AluOpType.mult
    IDENT = mybir.ActivationFunctionType.Identity

    nc = bacc.Bacc("TRN2", target_bir_lowering=False, debug=False)

    # z.T per core, tiled: [di, dt*TOK + t] = z[t, dt*P + di]
    z_d = nc.dram_tensor("z", [P, DT * TOK], f32r, kind="ExternalInput")
    # W per degree/rank-tile: [i, kt, di, dt*P + ki] = W[i, kt*P+ki, dt*P+di]
    w_d = nc.dram_tensor("w", [DEGREE, KT, P, DT * P], f32r, kind="ExternalInput")
    # C_w tiled: [ot, ki, kt*P + oi] = C_w[ot*P+oi, kt*P+ki]
    c_d = nc.dram_tensor("c", [OT, P, KT * P], f32r, kind="ExternalInput")
    # C_b tiled: [oi, ot] = C_b[ot*P + oi]
    cb_d = nc.dram_tensor("cb", [P, OT], f32, kind="ExternalInput")
    # x.T: [o, t]
    x_d = nc.dram_tensor("x", [O, TOK], f32, kind="ExternalOutput")

    z_ap, w_ap, c_ap, cb_ap, x_ap = (t.ap() for t in (z_d, w_d, c_d, cb_d, x_d))

    with tile.TileContext(nc) as tc, ExitStack() as ctx:
        zpool = ctx.enter_context(tc.tile_pool(name="z", bufs=DT))
        accpool = ctx.enter_context(tc.tile_pool(name="acc", bufs=KT))
        wpool = ctx.enter_context(tc.tile_pool(name="w", bufs=5))
        cbpool = ctx.enter_context(tc.tile_pool(name="cb", bufs=1))
        xpool = ctx.enter_context(tc.tile_pool(name="xt", bufs=4))
        pspool = ctx.enter_context(tc.tile_pool(name="ps", bufs=4, space="PSUM"))

        # Resident per-tile buffers: z.T (16x4KB/part) and acc (16x4KB/part).
        # Separate tiles give the scheduler fine-grained deps — readers of
        # z[dt] start as soon as that slice's DMA lands.
        z_sb = [zpool.tile([P, TOK], f32r, tag="z", name=f"z_sb{j}") for j in range(DT)]
        acc = [accpool.tile([P, TOK], f32r, tag="acc", name=f"acc{j}") for j in range(KT)]
        cb_sb = cbpool.tile([P, OT], f32)

        # Issue each z[dt] DMA lazily, right before the first group that
        # reads it — with the tril structure of degree 0 this streams z in
        # as the early rank-tile groups consume it instead of serializing
        # the whole 8MB load ahead of the first weight tile.
        z_issued = [False] * DT

        def ensure_z(lo_, hi_):
            for dt_ in range(lo_, hi_ + 1):
                if not z_issued[dt_]:
                    nc.gpsimd.dma_start(
                        z_sb[dt_][:], z_ap[:, dt_ * TOK : (dt_ + 1) * TOK]
                    )
                    z_issued[dt_] = True

        # Degree chain over acc[kt-block, tokens].
        for i in range(DEGREE):
            for kt in range(KT):
                rng = ranges[i][kt]
                if rng is None:
                    if i == 0:
                        # acc = mm = 0 for this rank block
                        nc.gpsimd.memset(acc[kt][:], 0.0)
                    continue
                lo, hi = rng
                ndt = hi - lo + 1
                ensure_z(lo, hi)
                w_sb = wpool.tile([P, ndt * P], f32r, tag="w")
                nc.sync.dma_start(w_sb[:], w_ap[i, kt][:, lo * P : (hi + 1) * P])
                ps = pspool.tile([P, TOK], f32)
                for tcx in range(TC):
                    for j, dt in enumerate(range(lo, hi + 1)):
                        nc.tensor.matmul(
                            ps[:, tcx * NC_CHUNK : (tcx + 1) * NC_CHUNK],
                            w_sb[:, j * P : (j + 1) * P],
                            z_sb[dt][:, tcx * NC_CHUNK : (tcx + 1) * NC_CHUNK],
                            start=(j == 0),
                            stop=(j == ndt - 1),
                        )
                dst = acc[kt][:]
                if i == 0:
                    nc.vector.tensor_copy(dst, ps[:])
                else:
                    # acc = (mm + 1) * acc  — one DVE op
                    nc.vector.scalar_tensor_tensor(dst, ps[:], 1.0, dst, ADD, MULT)

        # Final projection: x.T[ot-block] = C_w @ acc + C_b
        nc.sync.dma_start(cb_sb[:], cb_ap)
        for ot in range(OT):
            c_sb = wpool.tile([P, KT * P], f32r, tag="w")
            nc.sync.dma_start(c_sb[:], c_ap[ot])
            ps = pspool.tile([P, TOK], f32)
            for tcx in range(TC):
                for kt in range(KT):
                    nc.tensor.matmul(
                        ps[:, tcx * NC_CHUNK : (tcx + 1) * NC_CHUNK],
                        c_sb[:, kt * P : (kt + 1) * P],
                        acc[kt][:, tcx * NC_CHUNK : (tcx + 1) * NC_CHUNK],
                        start=(kt == 0),
                        stop=(kt == KT - 1),
                    )
            xt = xpool.tile([P, TOK], f32)
            nhalf = 2 if ot == OT - 1 else 1
            step = TOK // nhalf
            for h in range(nhalf):
                sl = slice(h * step, (h + 1) * step)
                nc.vector.tensor_scalar_add(xt[:, sl], ps[:, sl], cb_sb[:, ot : ot + 1])
                nc.gpsimd.dma_start(x_ap[ot * P : (ot + 1) * P, sl], xt[:, sl])

    nc.compile()
    return nc


def kernel(z, U, masks, C_w, C_b):
    from concourse.bass_utils import run_bass_kernel_spmd

    if os.environ.get("BASS_TRACE"):
        _install_ntff_shim()

    lead = z.shape[:-1]
    zf = np.ascontiguousarray(np.asarray(z, dtype=np.float32).reshape(-1, D))
    W = np.asarray(masks, dtype=np.float32) * np.asarray(U, dtype=np.float32)
    C_w = np.asarray(C_w, dtype=np.float32)
    C_b = np.asarray(C_b, dtype=np.float32)

    # Detect all-zero 128x128 blocks of W; build per-(degree, rank-tile)
    # contraction ranges. Only provably-zero blocks are skipped.
    blk = (
        np.abs(W.reshape(DEGREE, KT, P, DT, P)).max(axis=(2, 4)) > 0.0
    )  # [i, kt, dt]
    ranges = []
    for i in range(DEGREE):
        row = []
        for kt in range(KT):
            nz = np.flatnonzero(blk[i, kt])
            row.append((int(nz[0]), int(nz[-1])) if len(nz) else None)
        ranges.append(tuple(row))
    ranges = tuple(ranges)

    # Device layouts (see _build for index conventions).
    w_dev = np.ascontiguousarray(
        W.reshape(DEGREE, KT, P, DT, P).transpose(0, 1, 4, 3, 2)
    ).reshape(DEGREE, KT, P, DT * P)
    c_dev = np.ascontiguousarray(
        C_w.reshape(OT, P, KT, P).transpose(0, 3, 2, 1)
    ).reshape(OT, P, KT * P)
    cb_dev = np.ascontiguousarray(C_b.reshape(OT, P).T)

    in_maps = []
    for c in range(N_CORES):
        zs = zf[c * TOK : (c + 1) * TOK]  # [TOK, D]
        z_dev = np.ascontiguousarray(
            zs.T.reshape(DT, P, TOK).transpose(1, 0, 2)
        ).reshape(P, DT * TOK)
        in_maps.append({"z": z_dev, "w": w_dev, "c": c_dev, "cb": cb_dev})

    if _CACHE.get("ranges") != ranges:
        _CACHE["nc"] = _build(ranges)
        _CACHE["ranges"] = ranges
    nc = _CACHE["nc"]

    res = run_bass_kernel_spmd(nc, in_maps, core_ids=list(range(N_CORES)))
    _CACHE["last_result"] = res

    parts = [res.results[c]["x"].T for c in range(N_CORES)]  # each [TOK, O]
    x = np.concatenate(parts, axis=0)
    return x.reshape(*lead, O)
